# revision 42
# baseline (speedup 1.0000x reference)
"""Trainium2 Bass kernel for nn_Attention_64819646431478.

Single-layer causal attention, B=1, T=2048, DIM=1024, 16 heads, head_dim=64,
f32, with RMSNorm (eps=f32 eps) on Q and K heads.

Sharding: tensor-parallel over heads across 8 NeuronCores (2 heads/core).
Each core computes its heads' Q/K/V projections, causal attention, and the
partial output projection against its 128-row slice of w_o; the host sums
the 8 partial outputs (the "all-reduce" of the hint, done at gather time).

Per-core kernel layout choices:
  - Host passes x pre-transposed (xT [C, T]) and per-core weight slices
    pre-transposed, so every matmul contraction dim is on partitions.
  - Matmul inputs are bf16 (1 cyc/row PE, pipelined LDWEIGHTS; fp32 is 4x
    slower, fp32r 2x + serial weight loads); accumulation stays f32 in PSUM.
  - Scores are computed transposed: ST[tk, tq] = K @ Q^T per head, so the
    softmax reduction (over tk) is a matmul-with-ones. Because Q/K are
    RMS-normalized, |scores| <= 8, so exp needs no max-subtraction.
  - The softmax denominator is obtained free by appending a ones column to
    V in the PV matmul (lhsT = [V_h | 1], m=65: row 64 accumulates sums).
  - Reciprocals use DVE reciprocal_approx_fast (~51 ULP, one pass; the
    exact DVE RECIPROCAL measures ~13us/row-tensor). NOTE: that custom DVE
    op silently corrupts at base partition != 0 - operands get own tiles.
  - Normalize + w_o run per tq-chunk inside the attention loop, with the
    output DMA per (m-tile, chunk), so the tail fully overlaps.
  - w_o projection emitted transposed (out [m, t]); host transposes back.
"""

import os
import sys
import types

import numpy as np

# --- environment bootstrap (harness may run us from a bare directory) ---
for _p in ("/root/.axon_site", "/root/.axon_site/_ro/trn_rl_repo",
           "/root/.axon_site/_ro/pypackages", "/opt/trn_rl_repo"):
    if os.path.isdir(_p) and _p not in sys.path:
        sys.path.append(_p)


def _install_ntff_shim():
    """Provide antenv.axon_hooks (missing in this image) so trace=True works."""
    if "antenv.axon_hooks" in sys.modules:
        return
    mod = types.ModuleType("antenv.axon_hooks")
    mod._hook = None
    mod.set_axon_ntff_profile_hook = lambda h: setattr(mod, "_hook", h)
    mod.get_axon_ntff_profile_hook = lambda: mod._hook
    sys.modules["antenv.axon_hooks"] = mod
    try:
        import antenv
        antenv.axon_hooks = mod
        from trn_agent_boot.trn_boot import _ntff_profile_via_ctypes
        mod.set_axon_ntff_profile_hook(
            _ntff_profile_via_ctypes("/opt/axon/libaxon_pjrt.so"))
    except Exception:
        pass


_install_ntff_shim()

import ml_dtypes  # noqa: E402

import concourse.mybir as mybir  # noqa: E402
import concourse.tile as tile  # noqa: E402
from concourse import bacc  # noqa: E402

F32 = mybir.dt.float32
BF16 = mybir.dt.bfloat16
NP_BF16 = ml_dtypes.bfloat16
AF = mybir.ActivationFunctionType

T = 2048
C = 1024
D = 64
NCORES = 8
HPC = 2            # heads per core
JPC = HPC * D      # 128 j-columns per core
NTQ = 4            # tq chunks of 512
TQ = 512
NTK = 16           # tk tiles of 128
EPS = float(np.finfo(np.float32).eps)


def build_nc():
    nc = bacc.Bacc("TRN2", target_bir_lowering=False, debug=False,
                   num_devices=NCORES)

    xT_d = nc.dram_tensor("xT", [C, T], BF16, kind="ExternalInput")
    wqkv_d = nc.dram_tensor("wqkv", [C, 3 * JPC], BF16, kind="ExternalInput")
    wo_d = nc.dram_tensor("wo", [JPC, C], BF16, kind="ExternalInput")
    masks_d = nc.dram_tensor("masks", [4, 128, TQ], BF16, kind="ExternalInput")
    gq_d = nc.dram_tensor("gq", [2, 128], BF16, kind="ExternalInput")
    gk_d = nc.dram_tensor("gk", [2, 128], BF16, kind="ExternalInput")
    ones2_d = nc.dram_tensor("ones2", [2, 128], BF16, kind="ExternalInput")
    onescol_d = nc.dram_tensor("onescol", [128, 2], BF16, kind="ExternalInput")
    ident_d = nc.dram_tensor("ident", [128, 128], BF16, kind="ExternalInput")
    vones_d = nc.dram_tensor("vones", [128, 32], BF16, kind="ExternalInput")
    outT_d = nc.dram_tensor("outT", [C, T], F32, kind="ExternalOutput")

    with tile.TileContext(nc) as tc, nc.allow_low_precision("bf16 kernel"):
        from contextlib import ExitStack
        with ExitStack() as ctx:
            consts = ctx.enter_context(tc.tile_pool(name="consts", bufs=1))
            acts = ctx.enter_context(tc.tile_pool(name="acts", bufs=1))

            # ---- constants / inputs to SBUF ----
            wsb = consts.tile([128, 8, 3 * JPC], BF16)
            nc.gpsimd.dma_start(
                out=wsb[:], in_=wqkv_d.rearrange("(c p) j -> p c j", p=128))
            wo_sb = consts.tile([128, C], BF16)
            msb = consts.tile([128, 4, TQ], BF16)
            ident_sb = consts.tile([128, 128], BF16)
            gq_sb = consts.tile([2, 128], BF16)
            nc.gpsimd.dma_start(out=gq_sb[:], in_=gq_d[:])
            gk_sb = consts.tile([2, 128], BF16)
            nc.gpsimd.dma_start(out=gk_sb[:], in_=gk_d[:])
            oneh_sb = []
            for h in range(HPC):
                t_ = consts.tile([1, 128], BF16, name=f"oneh{h}")
                nc.gpsimd.dma_start(out=t_[:], in_=ones2_d[h:h + 1, :])
                oneh_sb.append(t_)
            onescol_sb = consts.tile([128, 2], BF16)
            nc.gpsimd.dma_start(out=onescol_sb[:], in_=onescol_d[:])

            # ---- persistent activations ----
            QTn = acts.tile([128, T], BF16)     # [ (h,d), t ] normalized Q^T
            KTn = acts.tile([128, T], BF16)
            V_sb = acts.tile([128, NTK, 130], BF16)  # [tk, r, (Vh0|1|Vh1|1)]
            ctx_un = acts.tile([128, T], BF16)  # unnormalized ctx^T
            ctxT = acts.tile([128, T], BF16)    # normalized ctx^T
            # NOTE: reciprocal_approx_fast (custom DVE op) only works at
            # base partition 0 -> every operand gets its own tile.
            rms_q = acts.tile([2, T], F32)
            rms_k = acts.tile([2, T], F32)
            rrf_q = acts.tile([2, T], F32)      # 1/rms (fp32, approx recip)
            rrf_k = acts.tile([2, T], F32)
            rec_q = acts.tile([2, T], BF16)     # rsqrt(mean q^2 + eps)
            rec_k = acts.tile([2, T], BF16)
            sg = [acts.tile([1, T], F32, name=f"sg{h}") for h in range(HPC)]
            sgf = [acts.tile([1, T], F32, name=f"sgf{h}") for h in range(HPC)]
            recs = [acts.tile([1, T], BF16, name=f"recs{h}")
                    for h in range(HPC)]

            # ones columns of V tiles (cols 64 and 129)
            vones_r = vones_d.rearrange("p (r u) -> p r u", u=2)
            nc.sync.dma_start(out=V_sb[:, :, 64:65], in_=vones_r[:, :, 0:1])
            nc.sync.dma_start(out=V_sb[:, :, 129:130],
                              in_=vones_r[:, :, 1:2])
            eps_sb = consts.tile([2, 1], F32)
            nc.vector.memset(eps_sb[:], EPS)

            # ================= Phase B: projections + RMSNorm ==============
            with (
                tc.tile_pool(name="xtp", bufs=1) as xtp,
                tc.tile_pool(name="sqp", bufs=3) as sqp,
                tc.tile_pool(name="ps_proj", bufs=3, space="PSUM") as ps_proj,
                tc.tile_pool(name="ps_sums", bufs=1, space="PSUM") as ps_sums,
                tc.tile_pool(name="ps_b", bufs=2, space="PSUM") as ps_b,
                tc.tile_pool(name="ps_tp", bufs=2, space="PSUM") as ps_tp,
            ):
                xT_sb = xtp.tile([128, 8, T], BF16)
                xT_r = xT_d.rearrange("(c p) t -> p c t", p=128)
                for ci in range(8):
                    # split xT across SWDGE and HWDGE queues; attention-only
                    # constants are issued after so they don't delay it
                    eng = nc.gpsimd if ci % 2 == 0 else nc.sync
                    eng.dma_start(out=xT_sb[:, ci, :], in_=xT_r[:, ci, :])
                nc.sync.dma_start(out=ident_sb[:], in_=ident_d[:])
                nc.sync.dma_start(out=msb[:],
                                  in_=masks_d.rearrange("s p f -> p s f"))
                nc.sync.dma_start(out=wo_sb[:], in_=wo_d[:])
                VT_sb = xtp.tile([128, T], BF16)   # [ j, t ] V^T
                QT_raw = xtp.tile([128, T], BF16)  # un-normalized Q^T
                KT_raw = xtp.tile([128, T], BF16)

                # Q/K/V per tq-chunk so attention can start on early chunks.
                # rsqrt = approx-recip(Sqrt): Square/Sqrt share one ACT
                # table set, reciprocal_approx_fast is one DVE pass.
                for c4 in range(NTQ):
                    sl = slice(TQ * c4, TQ * (c4 + 1))
                    for jbase, g2_sb, rms_sb, rrf_sb, rec_sb, raw, dst in (
                        (0, gq_sb, rms_q, rrf_q, rec_q, QT_raw, QTn),
                        (JPC, gk_sb, rms_k, rrf_k, rec_k, KT_raw, KTn),
                    ):
                        pp = ps_proj.tile([128, TQ], F32, tag="proj",
                                          name="pp")
                        for ci in range(8):
                            nc.tensor.matmul(
                                pp[:], wsb[:, ci, jbase:jbase + 128],
                                xT_sb[:, ci, sl],
                                start=(ci == 0), stop=(ci == 7))
                        sq = sqp.tile([128, TQ], BF16, tag="sq", name="sq")
                        nc.scalar.activation(sq[:], pp[:], AF.Square)
                        sums = ps_sums.tile([2, TQ], F32, tag="sums",
                                            name="sums")
                        nc.tensor.matmul(sums[:], onescol_sb[:], sq[:],
                                         start=True, stop=True)
                        nc.scalar.activation(rms_sb[:, sl], sums[:],
                                             AF.Sqrt, bias=eps_sb[:],
                                             scale=1.0 / D)
                        nc.vector.tensor_copy(raw[:, sl], pp[:])
                        nc.vector.reciprocal_approx_fast(
                            out=rrf_sb[:, sl], in_=rms_sb[:, sl])
                        nc.vector.tensor_copy(rec_sb[:, sl], rrf_sb[:, sl])
                        bb = ps_b.tile([128, TQ], F32, tag="b", name="bb")
                        nc.tensor.matmul(bb[:], g2_sb[:], rec_sb[:, sl],
                                         start=True, stop=True)
                        nc.vector.tensor_mul(dst[:, sl], raw[:, sl], bb[:])

                    # V^T projection chunk + PE-transpose into V_sb
                    pv = ps_proj.tile([128, TQ], F32, tag="proj", name="pv")
                    for ci in range(8):
                        nc.tensor.matmul(
                            pv[:], wsb[:, ci, 2 * JPC:3 * JPC],
                            xT_sb[:, ci, sl],
                            start=(ci == 0), stop=(ci == 7))
                    nc.vector.tensor_copy(VT_sb[:, sl], pv[:])
                    for r in range(4 * c4, 4 * c4 + 4):
                        tp = ps_tp.tile([128, 128], BF16, tag="tp",
                                        name=f"tp{r}")
                        nc.tensor.transpose(tp[:],
                                            VT_sb[:, 128 * r:128 * (r + 1)],
                                            ident_sb[:])
                        nc.vector.tensor_copy(V_sb[:, r, 0:64], tp[:, 0:64])
                        nc.vector.tensor_copy(V_sb[:, r, 65:129],
                                              tp[:, 64:128])

            # ========== Phase C: attention + normalize + w_o, per chunk ====
            outT_r = outT_d.rearrange("(m p) t -> p m t", p=128)
            with (
                tc.tile_pool(name="ep", bufs=6) as ep,
                tc.tile_pool(name="stgp", bufs=1) as stgp,
                tc.tile_pool(name="ps_st0", bufs=1, space="PSUM") as ps_st0,
                tc.tile_pool(name="ps_st1", bufs=1, space="PSUM") as ps_st1,
                tc.tile_pool(name="ps_ot", bufs=1, space="PSUM") as ps_ot,
                tc.tile_pool(name="ps_wrk", bufs=2, space="PSUM") as ps_wrk,
            ):
                st_pools = (ps_st0, ps_st1)
                stg_big = stgp.tile([128, 8, T], F32)   # w_o out staging
                for c4 in range(NTQ):
                    qsl = slice(TQ * c4, TQ * (c4 + 1))
                    n_tk = 4 * (c4 + 1)
                    ot = [ps_ot.tile([65, TQ], F32, tag=f"ot{h}",
                                     name=f"ot{h}_{c4}")
                          for h in range(HPC)]
                    for g in range(n_tk // 2):
                        for h in range(HPC):
                            hsl = slice(64 * h, 64 * (h + 1))
                            st = st_pools[h].tile([128, 2 * TQ], F32,
                                                  tag=f"st{h}",
                                                  name=f"st{h}")
                            for rl in range(2):
                                r = 2 * g + rl
                                nc.tensor.matmul(
                                    st[:, TQ * rl:TQ * (rl + 1)],
                                    KTn[hsl, 128 * r:128 * (r + 1)],
                                    QTn[hsl, qsl], start=True, stop=True)
                            e_t = ep.tile([128, 2 * TQ], BF16, tag=f"e{h}",
                                          name=f"e{h}")
                            nc.scalar.activation(e_t[:], st[:], AF.Exp,
                                                 scale=float(D) ** -0.5)
                            if 2 * g >= 4 * c4:  # diagonal band: apply
                                # masks; both slices in one DVE op (the two
                                # mask planes are adjacent in msb)
                                s0 = 2 * g - 4 * c4
                                ev = e_t[:, 0:2 * TQ].rearrange(
                                    "p (s f) -> p s f", f=TQ)
                                nc.vector.tensor_mul(
                                    ev, ev, msb[:, s0:s0 + 2, :])
                            for rl in range(2):
                                r = 2 * g + rl
                                nc.tensor.matmul(
                                    ot[h][:],
                                    V_sb[:, r, 65 * h:65 * (h + 1)],
                                    e_t[:, TQ * rl:TQ * (rl + 1)],
                                    start=(r == 0), stop=(r == n_tk - 1))
                    # stage ctx + softmax sums; normalize with approx recip
                    for h in range(HPC):
                        hsl = slice(64 * h, 64 * (h + 1))
                        nc.vector.tensor_copy(ctx_un[hsl, qsl],
                                              ot[h][0:64, :])
                        nc.vector.tensor_copy(sg[h][0:1, qsl],
                                              ot[h][64:65, :])
                        nc.vector.reciprocal_approx_fast(
                            out=sgf[h][0:1, qsl], in_=sg[h][0:1, qsl])
                        nc.vector.tensor_copy(recs[h][0:1, qsl],
                                              sgf[h][0:1, qsl])
                        b2 = ps_wrk.tile([128, TQ], F32, tag="wrk",
                                         name=f"b2{h}")
                        nc.tensor.matmul(b2[:], oneh_sb[h][:],
                                         recs[h][0:1, qsl],
                                         start=True, stop=True)
                        nc.vector.tensor_mul(ctxT[hsl, qsl],
                                             ctx_un[hsl, qsl], b2[hsl, :])
                    # w_o projection for this chunk; DMA out per (mu, chunk)
                    # so output transfers overlap the remaining attention
                    for mu in range(8):
                        wop = ps_wrk.tile([128, TQ], F32, tag="wrk",
                                          name=f"wop{mu}")
                        nc.tensor.matmul(wop[:],
                                         wo_sb[:, 128 * mu:128 * (mu + 1)],
                                         ctxT[:, qsl], start=True, stop=True)
                        nc.vector.tensor_copy(stg_big[:, mu, qsl], wop[:])
                        nc.sync.dma_start(out=outT_r[:, mu, qsl],
                                          in_=stg_big[:, mu, qsl])

    nc.compile()
    return nc


_NC_CACHE = None


def _get_nc():
    global _NC_CACHE
    if _NC_CACHE is None:
        _NC_CACHE = build_nc()
    return _NC_CACHE


def _make_in_maps(x, w_q, w_k, w_v, w_o, q_gamma, k_gamma):
    x = np.asarray(x, dtype=np.float32)
    xT = np.ascontiguousarray(x.reshape(T, C).T).astype(NP_BF16)  # [C, T]

    p = np.arange(128)
    f = np.arange(TQ)
    masks = np.zeros((4, 128, TQ), dtype=NP_BF16)
    for s in range(4):
        masks[s] = (f[None, :] >= (p[:, None] + 128 * s)).astype(NP_BF16)

    blk = (p[None, :] // 64 == np.arange(2)[:, None])      # [2, 128] bool
    gq = blk * np.tile(np.asarray(q_gamma, np.float32), 2)[None, :]
    gk = blk * np.tile(np.asarray(k_gamma, np.float32), 2)[None, :]
    ones2 = blk.astype(NP_BF16)
    onescol = np.ascontiguousarray(ones2.T)
    ident = np.eye(128, dtype=NP_BF16)

    common = dict(xT=xT, masks=masks,
                  gq=gq.astype(NP_BF16), gk=gk.astype(NP_BF16),
                  ones2=ones2, onescol=onescol, ident=ident,
                  vones=np.ones((128, 32), dtype=NP_BF16))

    in_maps = []
    for i in range(NCORES):
        rows = slice(JPC * i, JPC * (i + 1))
        wqkv = np.concatenate(
            [np.asarray(w_q, np.float32)[rows].T,
             np.asarray(w_k, np.float32)[rows].T,
             np.asarray(w_v, np.float32)[rows].T], axis=1)  # [C, 384]
        wo = np.asarray(w_o, np.float32)[:, rows].T          # [128, C]
        in_maps.append(dict(common,
                            wqkv=np.ascontiguousarray(wqkv).astype(NP_BF16),
                            wo=np.ascontiguousarray(wo).astype(NP_BF16)))
    return in_maps


def _run(x, w_q, w_k, w_v, w_o, q_gamma, k_gamma, trace=False):
    import time

    from concourse.bass_utils import run_bass_kernel_spmd
    nc = _get_nc()
    in_maps = _make_in_maps(x, w_q, w_k, w_v, w_o, q_gamma, k_gamma)
    res = None
    for attempt in range(3):
        try:
            res = run_bass_kernel_spmd(nc, in_maps, list(range(NCORES)),
                                       trace=trace)
            break
        except Exception:
            # rare transient NRT_EXEC_UNIT_UNRECOVERABLE under axon; the
            # terminal resets the device on the next load
            if attempt == 2:
                raise
            time.sleep(3.0)
    acc = np.zeros((C, T), dtype=np.float64)
    for r in res.results:
        acc += r["outT"].astype(np.float64)
    out = acc.T.astype(np.float32).reshape(1, T, C)
    return out, res


def kernel(x, w_q, w_k, w_v, w_o, q_gamma, k_gamma):
    out, _ = _run(x, w_q, w_k, w_v, w_o, q_gamma, k_gamma, trace=False)
    return out


# revision 43
# speedup vs baseline: 1.0897x; 1.0897x over previous
"""Trainium2 Bass kernel for nn_Attention_64819646431478.

Single-layer causal attention, B=1, T=2048, DIM=1024, 16 heads, head_dim=64,
f32, with RMSNorm (eps=f32 eps) on Q and K heads.

Sharding: tensor-parallel over heads across 8 NeuronCores (2 heads/core).
Each core computes its heads' Q/K/V projections, causal attention, and the
partial output projection against its 128-row slice of w_o; the host sums
the 8 partial outputs (the "all-reduce" of the hint, done at gather time).

Per-core kernel layout choices:
  - Host passes x pre-transposed (xT [C, T]) and per-core weight slices
    pre-transposed, so every matmul contraction dim is on partitions.
  - Matmul inputs are bf16 (1 cyc/row PE, pipelined LDWEIGHTS; fp32 is 4x
    slower, fp32r 2x + serial weight loads); accumulation stays f32 in PSUM.
  - Scores are computed transposed: ST[tk, tq] = K @ Q^T per head, so the
    softmax reduction (over tk) is a matmul-with-ones. Because Q/K are
    RMS-normalized, |scores| <= 8, so exp needs no max-subtraction.
  - The softmax denominator is obtained free by appending a ones column to
    V in the PV matmul (lhsT = [V_h | 1], m=65: row 64 accumulates sums).
  - Reciprocals use DVE reciprocal_approx_fast (~51 ULP, one pass; the
    exact DVE RECIPROCAL measures ~13us/row-tensor). NOTE: that custom DVE
    op silently corrupts at base partition != 0 - operands get own tiles.
  - Normalize + w_o run per tq-chunk inside the attention loop, with the
    output DMA per (m-tile, chunk), so the tail fully overlaps.
  - w_o projection emitted transposed (out [m, t]); host transposes back.
"""

import os
import sys
import types

import numpy as np

# --- environment bootstrap (harness may run us from a bare directory) ---
for _p in ("/root/.axon_site", "/root/.axon_site/_ro/trn_rl_repo",
           "/root/.axon_site/_ro/pypackages", "/opt/trn_rl_repo"):
    if os.path.isdir(_p) and _p not in sys.path:
        sys.path.append(_p)


def _install_ntff_shim():
    """Provide antenv.axon_hooks (missing in this image) so trace=True works."""
    if "antenv.axon_hooks" in sys.modules:
        return
    mod = types.ModuleType("antenv.axon_hooks")
    mod._hook = None
    mod.set_axon_ntff_profile_hook = lambda h: setattr(mod, "_hook", h)
    mod.get_axon_ntff_profile_hook = lambda: mod._hook
    sys.modules["antenv.axon_hooks"] = mod
    try:
        import antenv
        antenv.axon_hooks = mod
        from trn_agent_boot.trn_boot import _ntff_profile_via_ctypes
        mod.set_axon_ntff_profile_hook(
            _ntff_profile_via_ctypes("/opt/axon/libaxon_pjrt.so"))
    except Exception:
        pass


_install_ntff_shim()

import ml_dtypes  # noqa: E402

import concourse.mybir as mybir  # noqa: E402
import concourse.tile as tile  # noqa: E402
from concourse import bacc  # noqa: E402

F32 = mybir.dt.float32
BF16 = mybir.dt.bfloat16
NP_BF16 = ml_dtypes.bfloat16
AF = mybir.ActivationFunctionType

T = 2048
C = 1024
D = 64
NCORES = 8
HPC = 2            # heads per core
JPC = HPC * D      # 128 j-columns per core
NTQ = 4            # tq chunks of 512
TQ = 512
NTK = 16           # tk tiles of 128
EPS = float(np.finfo(np.float32).eps)


def build_nc():
    nc = bacc.Bacc("TRN2", target_bir_lowering=False, debug=False,
                   num_devices=NCORES)

    xT_d = nc.dram_tensor("xT", [C, T], BF16, kind="ExternalInput")
    wqkv_d = nc.dram_tensor("wqkv", [C, 3 * JPC], BF16, kind="ExternalInput")
    wo_d = nc.dram_tensor("wo", [JPC, C], BF16, kind="ExternalInput")
    masks_d = nc.dram_tensor("masks", [4, 128, TQ], BF16, kind="ExternalInput")
    gq_d = nc.dram_tensor("gq", [2, 128], BF16, kind="ExternalInput")
    gk_d = nc.dram_tensor("gk", [2, 128], BF16, kind="ExternalInput")
    ones2_d = nc.dram_tensor("ones2", [2, 128], BF16, kind="ExternalInput")
    onescol_d = nc.dram_tensor("onescol", [128, 2], BF16, kind="ExternalInput")
    ident_d = nc.dram_tensor("ident", [128, 128], BF16, kind="ExternalInput")
    vones_d = nc.dram_tensor("vones", [128, 32], BF16, kind="ExternalInput")
    outT_d = nc.dram_tensor("outT", [C, T], F32, kind="ExternalOutput")

    with tile.TileContext(nc) as tc, nc.allow_low_precision("bf16 kernel"):
        from contextlib import ExitStack
        with ExitStack() as ctx:
            consts = ctx.enter_context(tc.tile_pool(name="consts", bufs=1))
            acts = ctx.enter_context(tc.tile_pool(name="acts", bufs=1))

            # ---- constants / inputs to SBUF ----
            wsb = consts.tile([128, 8, 3 * JPC], BF16)
            nc.gpsimd.dma_start(
                out=wsb[:], in_=wqkv_d.rearrange("(c p) j -> p c j", p=128))
            wo_sb = consts.tile([128, C], BF16)
            nc.sync.dma_start(out=wo_sb[:], in_=wo_d[:])
            msb = consts.tile([128, 4, TQ], BF16)
            nc.sync.dma_start(out=msb[:],
                              in_=masks_d.rearrange("s p f -> p s f"))
            gq_sb = consts.tile([2, 128], BF16)
            nc.gpsimd.dma_start(out=gq_sb[:], in_=gq_d[:])
            gk_sb = consts.tile([2, 128], BF16)
            nc.gpsimd.dma_start(out=gk_sb[:], in_=gk_d[:])
            oneh_sb = []
            for h in range(HPC):
                t_ = consts.tile([1, 128], BF16, name=f"oneh{h}")
                nc.gpsimd.dma_start(out=t_[:], in_=ones2_d[h:h + 1, :])
                oneh_sb.append(t_)
            onescol_sb = consts.tile([128, 2], BF16)
            nc.gpsimd.dma_start(out=onescol_sb[:], in_=onescol_d[:])
            ident_sb = consts.tile([128, 128], BF16)
            nc.sync.dma_start(out=ident_sb[:], in_=ident_d[:])

            # ---- persistent activations ----
            QTn = acts.tile([128, T], BF16)     # [ (h,d), t ] normalized Q^T
            KTn = acts.tile([128, T], BF16)
            V_sb = acts.tile([128, NTK, 130], BF16)  # [tk, r, (Vh0|1|Vh1|1)]
            ctx_un = acts.tile([128, T], BF16)  # unnormalized ctx^T
            ctxT = acts.tile([128, T], BF16)    # normalized ctx^T
            # NOTE: reciprocal_approx_fast (custom DVE op) only works at
            # base partition 0 -> every operand gets its own tile.
            rms_q = acts.tile([2, T], F32)
            rms_k = acts.tile([2, T], F32)
            rrf_q = acts.tile([2, T], F32)      # 1/rms (fp32, approx recip)
            rrf_k = acts.tile([2, T], F32)
            rec_q = acts.tile([2, T], BF16)     # rsqrt(mean q^2 + eps)
            rec_k = acts.tile([2, T], BF16)
            sg = [acts.tile([1, T], F32, name=f"sg{h}") for h in range(HPC)]
            sgf = [acts.tile([1, T], F32, name=f"sgf{h}") for h in range(HPC)]
            recs = [acts.tile([1, T], BF16, name=f"recs{h}")
                    for h in range(HPC)]

            # ones columns of V tiles (cols 64 and 129)
            vones_r = vones_d.rearrange("p (r u) -> p r u", u=2)
            nc.sync.dma_start(out=V_sb[:, :, 64:65], in_=vones_r[:, :, 0:1])
            nc.sync.dma_start(out=V_sb[:, :, 129:130],
                              in_=vones_r[:, :, 1:2])
            eps_sb = consts.tile([2, 1], F32)
            nc.vector.memset(eps_sb[:], EPS)

            # ================= Phase B: projections + RMSNorm ==============
            with (
                tc.tile_pool(name="xtp", bufs=1) as xtp,
                tc.tile_pool(name="sqp", bufs=3) as sqp,
                tc.tile_pool(name="ps_proj", bufs=3, space="PSUM") as ps_proj,
                tc.tile_pool(name="ps_sums", bufs=1, space="PSUM") as ps_sums,
                tc.tile_pool(name="ps_b", bufs=2, space="PSUM") as ps_b,
                tc.tile_pool(name="ps_tp", bufs=2, space="PSUM") as ps_tp,
            ):
                xT_sb = xtp.tile([128, 8, T], BF16)
                xT_r = xT_d.rearrange("(c p) t -> p c t", p=128)
                for ci in range(8):
                    nc.gpsimd.dma_start(out=xT_sb[:, ci, :],
                                        in_=xT_r[:, ci, :])
                VT_sb = xtp.tile([128, T], BF16)   # [ j, t ] V^T
                QT_raw = xtp.tile([128, T], BF16)  # un-normalized Q^T
                KT_raw = xtp.tile([128, T], BF16)

                # Q/K/V per tq-chunk so attention can start on early chunks.
                # rsqrt = approx-recip(Sqrt): Square/Sqrt share one ACT
                # table set, reciprocal_approx_fast is one DVE pass.
                for c4 in range(NTQ):
                    sl = slice(TQ * c4, TQ * (c4 + 1))
                    for jbase, g2_sb, rms_sb, rrf_sb, rec_sb, raw, dst in (
                        (0, gq_sb, rms_q, rrf_q, rec_q, QT_raw, QTn),
                        (JPC, gk_sb, rms_k, rrf_k, rec_k, KT_raw, KTn),
                    ):
                        pp = ps_proj.tile([128, TQ], F32, tag="proj",
                                          name="pp")
                        for ci in range(8):
                            nc.tensor.matmul(
                                pp[:], wsb[:, ci, jbase:jbase + 128],
                                xT_sb[:, ci, sl],
                                start=(ci == 0), stop=(ci == 7))
                        sq = sqp.tile([128, TQ], BF16, tag="sq", name="sq")
                        nc.scalar.activation(sq[:], pp[:], AF.Square)
                        sums = ps_sums.tile([2, TQ], F32, tag="sums",
                                            name="sums")
                        nc.tensor.matmul(sums[:], onescol_sb[:], sq[:],
                                         start=True, stop=True)
                        nc.scalar.activation(rms_sb[:, sl], sums[:],
                                             AF.Sqrt, bias=eps_sb[:],
                                             scale=1.0 / D)
                        nc.vector.tensor_copy(raw[:, sl], pp[:])
                        nc.vector.reciprocal_approx_fast(
                            out=rrf_sb[:, sl], in_=rms_sb[:, sl])
                        nc.vector.tensor_copy(rec_sb[:, sl], rrf_sb[:, sl])
                        bb = ps_b.tile([128, TQ], F32, tag="b", name="bb")
                        nc.tensor.matmul(bb[:], g2_sb[:], rec_sb[:, sl],
                                         start=True, stop=True)
                        nc.vector.tensor_mul(dst[:, sl], raw[:, sl], bb[:])

                    # V^T projection chunk + PE-transpose into V_sb
                    pv = ps_proj.tile([128, TQ], F32, tag="proj", name="pv")
                    for ci in range(8):
                        nc.tensor.matmul(
                            pv[:], wsb[:, ci, 2 * JPC:3 * JPC],
                            xT_sb[:, ci, sl],
                            start=(ci == 0), stop=(ci == 7))
                    nc.vector.tensor_copy(VT_sb[:, sl], pv[:])
                    for r in range(4 * c4, 4 * c4 + 4):
                        tp = ps_tp.tile([128, 128], BF16, tag="tp",
                                        name=f"tp{r}")
                        nc.tensor.transpose(tp[:],
                                            VT_sb[:, 128 * r:128 * (r + 1)],
                                            ident_sb[:])
                        nc.vector.tensor_copy(V_sb[:, r, 0:64], tp[:, 0:64])
                        nc.vector.tensor_copy(V_sb[:, r, 65:129],
                                              tp[:, 64:128])

            # ========== Phase C: attention + normalize + w_o, per chunk ====
            outT_r = outT_d.rearrange("(m p) t -> p m t", p=128)
            with (
                tc.tile_pool(name="ep", bufs=6) as ep,
                tc.tile_pool(name="stgp", bufs=1) as stgp,
                tc.tile_pool(name="ps_st0", bufs=1, space="PSUM") as ps_st0,
                tc.tile_pool(name="ps_st1", bufs=1, space="PSUM") as ps_st1,
                tc.tile_pool(name="ps_ot", bufs=1, space="PSUM") as ps_ot,
                tc.tile_pool(name="ps_wrk", bufs=2, space="PSUM") as ps_wrk,
            ):
                st_pools = (ps_st0, ps_st1)
                stg_big = stgp.tile([128, 8, T], F32)   # w_o out staging
                for c4 in range(NTQ):
                    qsl = slice(TQ * c4, TQ * (c4 + 1))
                    n_tk = 4 * (c4 + 1)
                    ot = [ps_ot.tile([65, TQ], F32, tag=f"ot{h}",
                                     name=f"ot{h}_{c4}")
                          for h in range(HPC)]
                    for g in range(n_tk // 2):
                        for h in range(HPC):
                            hsl = slice(64 * h, 64 * (h + 1))
                            st = st_pools[h].tile([128, 2 * TQ], F32,
                                                  tag=f"st{h}",
                                                  name=f"st{h}")
                            for rl in range(2):
                                r = 2 * g + rl
                                nc.tensor.matmul(
                                    st[:, TQ * rl:TQ * (rl + 1)],
                                    KTn[hsl, 128 * r:128 * (r + 1)],
                                    QTn[hsl, qsl], start=True, stop=True)
                            e_t = ep.tile([128, 2 * TQ], BF16, tag=f"e{h}",
                                          name=f"e{h}")
                            nc.scalar.activation(e_t[:], st[:], AF.Exp,
                                                 scale=float(D) ** -0.5)
                            if 2 * g >= 4 * c4:  # diagonal band: apply
                                # masks; both slices in one DVE op (the two
                                # mask planes are adjacent in msb)
                                s0 = 2 * g - 4 * c4
                                ev = e_t[:, 0:2 * TQ].rearrange(
                                    "p (s f) -> p s f", f=TQ)
                                nc.vector.tensor_mul(
                                    ev, ev, msb[:, s0:s0 + 2, :])
                            for rl in range(2):
                                r = 2 * g + rl
                                nc.tensor.matmul(
                                    ot[h][:],
                                    V_sb[:, r, 65 * h:65 * (h + 1)],
                                    e_t[:, TQ * rl:TQ * (rl + 1)],
                                    start=(r == 0), stop=(r == n_tk - 1))
                    # stage ctx + softmax sums; normalize with approx recip
                    for h in range(HPC):
                        hsl = slice(64 * h, 64 * (h + 1))
                        nc.vector.tensor_copy(ctx_un[hsl, qsl],
                                              ot[h][0:64, :])
                        nc.vector.tensor_copy(sg[h][0:1, qsl],
                                              ot[h][64:65, :])
                        nc.vector.reciprocal_approx_fast(
                            out=sgf[h][0:1, qsl], in_=sg[h][0:1, qsl])
                        nc.vector.tensor_copy(recs[h][0:1, qsl],
                                              sgf[h][0:1, qsl])
                        b2 = ps_wrk.tile([128, TQ], F32, tag="wrk",
                                         name=f"b2{h}")
                        nc.tensor.matmul(b2[:], oneh_sb[h][:],
                                         recs[h][0:1, qsl],
                                         start=True, stop=True)
                        nc.vector.tensor_mul(ctxT[hsl, qsl],
                                             ctx_un[hsl, qsl], b2[hsl, :])
                    # w_o projection for this chunk; DMA out per (mu, chunk)
                    # so output transfers overlap the remaining attention
                    for mu in range(8):
                        wop = ps_wrk.tile([128, TQ], F32, tag="wrk",
                                          name=f"wop{mu}")
                        nc.tensor.matmul(wop[:],
                                         wo_sb[:, 128 * mu:128 * (mu + 1)],
                                         ctxT[:, qsl], start=True, stop=True)
                        nc.vector.tensor_copy(stg_big[:, mu, qsl], wop[:])
                        nc.sync.dma_start(out=outT_r[:, mu, qsl],
                                          in_=stg_big[:, mu, qsl])

    nc.compile()
    return nc


_NC_CACHE = None


def _get_nc():
    global _NC_CACHE
    if _NC_CACHE is None:
        _NC_CACHE = build_nc()
    return _NC_CACHE


def _make_in_maps(x, w_q, w_k, w_v, w_o, q_gamma, k_gamma):
    x = np.asarray(x, dtype=np.float32)
    xT = np.ascontiguousarray(x.reshape(T, C).T).astype(NP_BF16)  # [C, T]

    p = np.arange(128)
    f = np.arange(TQ)
    masks = np.zeros((4, 128, TQ), dtype=NP_BF16)
    for s in range(4):
        masks[s] = (f[None, :] >= (p[:, None] + 128 * s)).astype(NP_BF16)

    blk = (p[None, :] // 64 == np.arange(2)[:, None])      # [2, 128] bool
    gq = blk * np.tile(np.asarray(q_gamma, np.float32), 2)[None, :]
    gk = blk * np.tile(np.asarray(k_gamma, np.float32), 2)[None, :]
    ones2 = blk.astype(NP_BF16)
    onescol = np.ascontiguousarray(ones2.T)
    ident = np.eye(128, dtype=NP_BF16)

    common = dict(xT=xT, masks=masks,
                  gq=gq.astype(NP_BF16), gk=gk.astype(NP_BF16),
                  ones2=ones2, onescol=onescol, ident=ident,
                  vones=np.ones((128, 32), dtype=NP_BF16))

    in_maps = []
    for i in range(NCORES):
        rows = slice(JPC * i, JPC * (i + 1))
        wqkv = np.concatenate(
            [np.asarray(w_q, np.float32)[rows].T,
             np.asarray(w_k, np.float32)[rows].T,
             np.asarray(w_v, np.float32)[rows].T], axis=1)  # [C, 384]
        wo = np.asarray(w_o, np.float32)[:, rows].T          # [128, C]
        in_maps.append(dict(common,
                            wqkv=np.ascontiguousarray(wqkv).astype(NP_BF16),
                            wo=np.ascontiguousarray(wo).astype(NP_BF16)))
    return in_maps


def _run(x, w_q, w_k, w_v, w_o, q_gamma, k_gamma, trace=False):
    import time

    from concourse.bass_utils import run_bass_kernel_spmd
    nc = _get_nc()
    in_maps = _make_in_maps(x, w_q, w_k, w_v, w_o, q_gamma, k_gamma)
    res = None
    for attempt in range(3):
        try:
            res = run_bass_kernel_spmd(nc, in_maps, list(range(NCORES)),
                                       trace=trace)
            break
        except Exception:
            # rare transient NRT_EXEC_UNIT_UNRECOVERABLE under axon; the
            # terminal resets the device on the next load
            if attempt == 2:
                raise
            time.sleep(3.0)
    acc = np.zeros((C, T), dtype=np.float64)
    for r in res.results:
        acc += r["outT"].astype(np.float64)
    out = acc.T.astype(np.float32).reshape(1, T, C)
    return out, res


def kernel(x, w_q, w_k, w_v, w_o, q_gamma, k_gamma):
    out, _ = _run(x, w_q, w_k, w_v, w_o, q_gamma, k_gamma, trace=False)
    return out
